# revision 1
# baseline (speedup 1.0000x reference)
"""Trainium2 Bass kernel for nn_CLARM_56693568307877.

Computes, for feature sets A [64,640,14,14] and B [128,640,14,14] and a QKV
projection W [240,640]:
    q,k,v = split(x^T W^T); S = q_b k_a^T / sqrt(80); P = softmax(S)
    rec = P v_a;  sim[b,a] = -||v_b - rec||^2_F
Output [128, 64] fp32.

Sharding: data-parallel over the b batch (16 per core x 8 cores);
features_a / W replicated. Everything device-side runs in bf16 with fp32
accumulation (validated: max rel err ~3e-4 vs fp32 reference).

Per-core device pipeline (B=16 b's, A=64 a's, N=M=196 tokens, D=80):
  phase 1: QKV projections on PE (W^T as stationary weights), d-major
           outputs; v_a additionally DMA-transposed to n-major for the
           second attention matmul.
  phase 2: per (a, 4-b subgroup): S^T = k_a^T.T @ q_b^T on PE -> PSUM,
           exp on ACT -> SBUF bf16, U = [v_a|1]^T @ expS^T on PE
           (row 80 = softmax denominator), egress PSUM->SBUF split
           between ACT and DVE.  Denominator rows are DMA-gathered
           across a 4-a wave into a [64,196] tile, inverted with the
           DVE reciprocal, broadcast back via a zero-step DMA, then
           rec = U*w, D = rec - v_b, D^2 and a segmented reduce give
           per-partition sums; a final (-1)-vector matmul reduces over
           partitions into -sum(D^2).
PSUM is managed as a ring of 8 one-bank slots shared by all phases.

Note: this walrus build accepts at most one semaphore wait per
instruction, rejects the custom-DVE ops and InstTensorTensorReduce,
and the xbar DMA-transpose corrupts non-zero-offset destinations;
_split_multi_waits and the scratch-tile transposes work around this.
"""

import numpy as np
import ml_dtypes

import concourse.bass as bass
import concourse.tile as tile
from concourse import mybir
from concourse.bass_utils import run_bass_kernel_spmd

BF16 = mybir.dt.bfloat16
F32 = mybir.dt.float32

NCORES = 8
A_FULL = 64
B_FULL = 128
HID = 640
KC = HID // 128  # 5
N = 196          # tokens (14*14)
D = 80           # inner dim
MPAD = 256       # m padded to 2*128 for clean matmul chunks
SCALE = 1.0 / np.sqrt(D)

_PROGRAM_CACHE = {}


def _build(Asz, Bsz):
    """Emit the Bass program for one core handling Bsz b's x Asz a's."""
    assert Bsz % 4 == 0 and Asz % 4 == 0
    NSG = Bsz // 4            # 4-b subgroups per a
    SG_WAVE = 4 * NSG         # subgroups per 4-a wave
    PW = 4 * Bsz              # pairs per wave

    nc = bass.Bass("TRN2", debug=False)
    fa = nc.dram_tensor("fa", [Asz, KC, 128, N], BF16, kind="ExternalInput")
    fb = nc.dram_tensor("fb", [Bsz, KC, 128, N], BF16, kind="ExternalInput")
    wt = nc.dram_tensor("wt", [KC, 128, 240], BF16, kind="ExternalInput")
    simo = nc.dram_tensor("sim", [Asz, Bsz], F32, kind="ExternalOutput")

    Exp = mybir.ActivationFunctionType.Exp
    mult = mybir.AluOpType.mult
    sub = mybir.AluOpType.subtract
    addop = mybir.AluOpType.add

    with tile.TileContext(nc) as tc:
        with (
            tc.tile_pool(name="const", bufs=1) as cpool,
            tc.tile_pool(name="ring", bufs=1, space="PSUM") as rpool,
            tc.tile_pool(name="x", bufs=3) as x_pool,
            tc.tile_pool(name="vt", bufs=3) as vt_pool,
            tc.tile_pool(name="e", bufs=6) as e_pool,
            tc.tile_pool(name="u", bufs=40) as u_pool,
            tc.tile_pool(name="wbt", bufs=8) as wb_pool,
            tc.tile_pool(name="rec", bufs=4) as rec_pool,
            tc.tile_pool(name="d", bufs=4) as d_pool,
            tc.tile_pool(name="scr", bufs=2) as scr_pool,
            tc.tile_pool(name="stg", bufs=8) as stg_pool,
            tc.tile_pool(name="wave", bufs=2) as wv_pool,
        ):
            wt_sb = cpool.tile([128, KC, 240], BF16, tag="wt")
            kT_all = cpool.tile([128, Asz, MPAD], BF16, tag="kT")
            vaug = cpool.tile([128, Asz, 2, 81], BF16, tag="vaug")
            qT_all = cpool.tile([128, Bsz, N], BF16, tag="qT")
            vbT_all = cpool.tile([80, Bsz, N], BF16, tag="vbT")
            ones_c = cpool.tile([128, 1], F32, tag="ones")
            ones_b = cpool.tile([1, 80], BF16, tag="onesb")
            ring = rpool.tile([128, 8, 512], F32, tag="ring")

            # one-time init
            nc.sync.dma_start(wt_sb, wt.ap().rearrange("k p c -> p k c"))
            nc.gpsimd.memset(kT_all[:], 0.0)
            nc.gpsimd.memset(qT_all[:], 0.0)
            nc.gpsimd.memset(vaug[:], 0.0)
            nc.gpsimd.memset(ones_c[:], 0.0)
            # -1 weights: the final ones-matmul then yields -sum(D^2) directly
            nc.gpsimd.memset(ones_c[0:80, :], -1.0)
            nc.gpsimd.memset(ones_b[:], 1.0)
            nc.gpsimd.memset(vaug[0:128, :, 0, 80:81], 1.0)
            nc.gpsimd.memset(vaug[0:68, :, 1, 80:81], 1.0)

            rp = [0]

            def rslot(k=1):
                s = rp[0] % 8
                rp[0] += k
                return s

            def qkv_batch(src, idx, want):
                """want: 'a' -> (k,v), 'b' -> (q,v); returns dict of psum APs."""
                xt = x_pool.tile([128, KC, N], BF16, tag="x")
                nc.sync.dma_start(xt, src[idx].rearrange("k p n -> p k n"))
                outs = {}
                cols = (("k", 80), ("v", 160)) if want == "a" else (("q", 0), ("v", 160))
                for name, c0 in cols:
                    s = rslot()
                    ps = ring[0:80, s, 0:N]
                    for kc in range(KC):
                        nc.tensor.matmul(
                            ps,
                            wt_sb[:, kc, c0:c0 + 80],
                            xt[:, kc, :],
                            start=(kc == 0),
                            stop=(kc == KC - 1),
                        )
                    outs[name] = ps
                return outs

            # phase 1, b batches
            for b in range(Bsz):
                o = qkv_batch(fb, b, "b")
                nc.scalar.copy(qT_all[0:80, b, :], o["q"])
                nc.scalar.copy(vbT_all[0:80, b, :], o["v"])

            wave_u = []   # (u_sb, sgb) for the current wave
            wave_a0 = 0
            pending = None  # deferred back-half of the previous subgroup

            def _flush_sg(p):
                es, u0, pa, pwidx, psgb, pden = p
                for kc in range(2):
                    for ncx in range(2):
                        nc.tensor.matmul(
                            ring[0:81, u0 + ncx, 0:392],
                            vaug[:, pa, kc, :],
                            es[kc][:, ncx, :],
                            start=(kc == 0),
                            stop=(kc == 1),
                        )
                u_sb = u_pool.tile([81, 2, 392], BF16, tag="u")
                nc.scalar.copy(u_sb[:, 0, :], ring[0:81, u0, 0:392])
                nc.vector.tensor_copy(u_sb[:, 1, :], ring[0:81, u0 + 1, 0:392])
                nc.sync.dma_start(
                    pden[pwidx * 4:(pwidx + 1) * 4, :], u_sb[80:81, :, :]
                )
                wave_u.append((u_sb, psgb))

            for a in range(Asz):
                if a % 4 == 0:
                    wave_a0 = a
                    den_coll = wv_pool.tile([PW, N], BF16, tag="den")
                # phase 1 for this a
                o = qkv_batch(fa, a, "a")
                nc.scalar.copy(kT_all[0:80, a, 0:N], o["k"])
                # vt padded to 256 cols so both DMA transposes are x128 wide;
                # pad cols are zeroed so vaug chunk-1 pad rows stay zero.
                vt = vt_pool.tile([80, MPAD], BF16, tag="vt")
                nc.gpsimd.memset(vt[:, N:MPAD], 0.0)
                nc.scalar.copy(vt[:, 0:N], o["v"])
                # the xbar transpose mangles data at non-zero dst offsets, so
                # transpose into offset-0 scratch tiles and copy into vaug
                t0 = vt_pool.tile([128, 80], BF16, tag="vtr0")
                t1 = vt_pool.tile([128, 80], BF16, tag="vtr1")
                nc.sync.dma_start_transpose(t0, vt[:, 0:128])
                nc.sync.dma_start_transpose(t1, vt[:, 128:MPAD])
                nc.vector.tensor_copy(vaug[0:128, a, 0, 0:80], t0)
                nc.vector.tensor_copy(vaug[0:68, a, 1, 0:80], t1[0:68, :])

                for sgb in range(NSG):
                    b0 = 4 * sgb
                    widx = (a % 4) * NSG + sgb  # subgroup index in wave
                    # front half: mm1 + exp; ring slots for U reserved now
                    sbank = []
                    for mc in range(2):
                        s0 = rslot(2)
                        assert s0 % 2 == 0
                        sbank.append(s0)
                        for ncx in range(2):
                            nc.tensor.matmul(
                                ring[:, s0 + ncx, 0:392],
                                kT_all[:, a, mc * 128:(mc + 1) * 128],
                                qT_all[:, b0 + 2 * ncx: b0 + 2 * ncx + 2, :],
                                start=True,
                                stop=True,
                            )
                    es = []
                    for mc in range(2):
                        e = e_pool.tile([128, 2, 392], BF16, tag="e")
                        nc.scalar.activation(
                            e, ring[:, sbank[mc]:sbank[mc] + 2, 0:392], Exp
                        )
                        es.append(e)
                    u0 = rslot(2)
                    assert u0 % 2 == 0
                    # back half of the PREVIOUS subgroup is emitted here so
                    # the PE can run this sg's mm1 while exp(prev) finishes
                    if pending is not None:
                        _flush_sg(pending)
                    pending = (es, u0, a, widx, sgb, den_coll)

                if a % 4 == 3:
                    if pending is not None:
                        _flush_sg(pending)
                        pending = None
                    # wave tail: reciprocal + broadcast + rec/D/reduce
                    den_f = wv_pool.tile([PW, N], F32, tag="denf")
                    nc.vector.tensor_copy(den_f, den_coll)
                    w_f = wv_pool.tile([PW, N], F32, tag="wf")
                    nc.vector.reciprocal(w_f, den_f)
                    w_b = wv_pool.tile([PW, N], BF16, tag="wb")
                    nc.vector.tensor_copy(w_b, w_f)
                    simcol = wv_pool.tile([128, PW], F32, tag="sc")
                    nc.vector.memset(simcol[:], 0.0)
                    for wi, (u_sb, sgb) in enumerate(wave_u):
                        wb_t = wb_pool.tile([80, 4, N], BF16, tag="wbt")
                        stg = stg_pool.tile([1, 4, N], BF16, tag="stg")
                        nc.sync.dma_start(stg, w_b[wi * 4:(wi + 1) * 4, :])
                        src_b = bass.AP(
                            stg.tensor, stg.offset, [[1, 1], [0, 80], [1, 4 * N]]
                        )
                        nc.sync.dma_start(wb_t, src_b)
                        rec = rec_pool.tile([80, 2, 392], BF16, tag="rec")
                        nc.vector.tensor_tensor(rec, u_sb[0:80, :, :], wb_t, op=mult)
                        d_t = d_pool.tile([80, 2, 392], BF16, tag="d")
                        nc.vector.tensor_tensor(
                            d_t, rec, vbT_all[:, 4 * sgb:4 * sgb + 4, :], op=sub
                        )
                        d2 = scr_pool.tile([80, 4, N], BF16, tag="scr")
                        dv = d_t.rearrange("p c x -> p (c x)").rearrange(
                            "p (j n) -> p j n", j=4
                        )
                        nc.vector.tensor_tensor(d2, dv, dv, op=mult)
                        nc.vector.reduce_sum(
                            out=simcol[0:80, wi * 4:(wi + 1) * 4],
                            in_=d2,
                            axis=mybir.AxisListType.X,
                        )
                    wave_u = []
                    # reduce over the 80 d-partitions with a ones matmul
                    cs = rslot(2)
                    nc.tensor.matmul(
                        ring[0:1, cs, 0:PW], ones_c[:, 0:1], simcol[:],
                        start=True, stop=True,
                    )
                    simrow = wv_pool.tile([1, PW], F32, tag="sr")
                    nc.vector.tensor_copy(simrow, ring[0:1, cs, 0:PW])
                    # simrow layout [a_local, b] matches simo rows wave_a0..+4
                    nc.sync.dma_start(simo[wave_a0:wave_a0 + 4, :], simrow[0:1, :])

    return nc


def _split_multi_waits(nc):
    """This walrus build accepts at most one semaphore wait per instruction;
    Tile emits several (incl. its tail drain). Hoist extra waits onto
    single-wait engine NoOps inserted just before the instruction."""
    cnt = 0
    for f in nc.m.functions:
        for bb in f.blocks:
            insts = list(bb.instructions)
            out = []
            changed = False
            for inst in insts:
                si = getattr(inst, "sync_info", None)
                ws = list(si.on_wait) if (si is not None and si.on_wait) else []
                if len(ws) > 1:
                    changed = True
                    for w in ws[:-1]:
                        cnt += 1
                        out.append(mybir.InstNoOp(
                            name=f"WSPLIT-{cnt}",
                            engine=inst.engine,
                            ins=[], outs=[],
                            sync_info=mybir.SyncInfo(on_wait=[w], on_update=[]),
                        ))
                    si.on_wait = [ws[-1]]
                    inst.sync_info = si
                out.append(inst)
            if changed:
                bb.instructions = out
    return nc


def _get_program(Asz, Bsz):
    key = (Asz, Bsz)
    if key not in _PROGRAM_CACHE:
        _PROGRAM_CACHE[key] = _split_multi_waits(_build(Asz, Bsz))
    return _PROGRAM_CACHE[key]


def _prep_inputs(features_a, features_b, W_qkv, Asz, Bsz, ncores):
    """Host-side: cast to bf16, fold the 1/sqrt(D) scale into Wq, reshape."""
    fa = features_a.reshape(Asz, HID, N).astype(ml_dtypes.bfloat16)
    fa = fa.reshape(Asz, KC, 128, N)
    wt = W_qkv.T.copy().astype(np.float32)   # [640, 240]
    wt[:, 0:D] *= SCALE
    wt = wt.astype(ml_dtypes.bfloat16).reshape(KC, 128, 240)
    fbs = []
    for c in range(ncores):
        fb = features_b[c * Bsz:(c + 1) * Bsz].reshape(Bsz, HID, N)
        fb = fb.astype(ml_dtypes.bfloat16).reshape(Bsz, KC, 128, N)
        fbs.append(fb)
    return fa, fbs, wt


def kernel(features_a, features_b, W_qkv):
    Asz = features_a.shape[0]
    Bfull = features_b.shape[0]
    ncores = NCORES
    Bsz = Bfull // ncores
    fa, fbs, wt = _prep_inputs(
        np.asarray(features_a), np.asarray(features_b), np.asarray(W_qkv),
        Asz, Bsz, ncores,
    )
    nc = _get_program(Asz, Bsz)
    in_maps = [{"fa": fa, "fb": fbs[c], "wt": wt} for c in range(ncores)]
    res = run_bass_kernel_spmd(nc, in_maps, core_ids=list(range(ncores)))
    out = np.concatenate([res.results[c]["sim"].T for c in range(ncores)], axis=0)
    return out.astype(np.float32)



# revision 31
# speedup vs baseline: 2.1272x; 2.1272x over previous
"""Trainium2 Bass kernel for nn_CLARM_56693568307877.

Computes, for feature sets A [64,640,14,14] and B [128,640,14,14] and a QKV
projection W [240,640]:
    q,k,v = split(x^T W^T); S = q_b k_a^T / sqrt(80); P = softmax(S)
    rec = P v_a;  sim[b,a] = -||v_b - rec||^2_F
Output [128, 64] fp32.

Sharding: data-parallel over the b batch (16 per core x 8 cores);
features_a / W replicated. Device math in bf16 with fp32 accumulation
(max rel err ~3.4e-4 vs the fp32 reference).

Design (per-core B=16, A=64, N=196 tokens, D=80), n-major attention tail:
  phase 1 (one psum slot per batch, outside the subgroup windows):
    QKV on PE (W^T stationary, q|k and v packed into one bank), one ACT
    copy out d-major; v is PE-transposed (identity matmul, bf16 psum
    views packed in the same bank) into n-major chunk tiles
    va_cat/vb_cat [128/68 rows, chunk, batch, 81] with ones at col 80.
  phase 2, per (a, 4-b subgroup) on two alternating 4-slot psum windows:
    mm1  S^T chunks [mchunk, 392] for 4 pairs -> the window's 4 slots
    exp  one ACT instruction per m-chunk -> es bf16 SBUF
    mm2  U'[n, 4x81] = es^T @ [v_a|1] per n-chunk (den lands in col 80)
    tail w = 1/den (DVE, free-4); rec = U'*w via a stride-0 broadcast
         multiply (doubles as the psum egress, DVE); D = rec - vbn
         (DVE, 2x); D^2 on Pool; -sum over n via a (-1)-stationary
         matmul into a spare slot (row 0) of the own window; sum over d
         with one partition-1 reduce straight into the output row.
  The subgroup stages run as a 5-deep software pipeline (mm1+exp | mm2 +
  reciprocal/normalize, per n-chunk, sandwiched between the mm1 halves |
  subtract+square | n-reduce+d-reduce), so every engine's in-order queue
  only ever holds work whose inputs are already available, and the psum
  window of subgroup g is recycled by mm1(g+2) right after normalize(g).

Note: this walrus build accepts at most one semaphore wait per
instruction; _split_multi_waits hoists extras onto NoOps.
"""

import numpy as np
import ml_dtypes

import concourse.bass as bass
import concourse.tile as tile
from concourse import mybir
from concourse.bass_utils import run_bass_kernel_spmd

BF16 = mybir.dt.bfloat16
F32 = mybir.dt.float32

NCORES = 8
A_FULL = 64
B_FULL = 128
HID = 640
KC = HID // 128  # 5
N = 196          # tokens (14*14)
D = 80           # inner dim
MPAD = 256       # m padded to 2*128 so mm1 writes full psum rows
SCALE = 1.0 / np.sqrt(D)

_PROGRAM_CACHE = {}


def _build(Asz, Bsz):
    """Emit the Bass program for one core handling Bsz b's x Asz a's."""
    assert Bsz % 4 == 0 and Asz % 4 == 0
    NSG = Bsz // 4            # 4-b subgroups per a

    nc = bass.Bass("TRN2", debug=False)
    fa = nc.dram_tensor("fa", [Asz, KC, 128, N], BF16, kind="ExternalInput")
    fb = nc.dram_tensor("fb", [Bsz, KC, 128, N], BF16, kind="ExternalInput")
    wt = nc.dram_tensor("wt", [KC, 128, 240], BF16, kind="ExternalInput")
    eye = nc.dram_tensor("eye", [128, 128], BF16, kind="ExternalInput")
    simo = nc.dram_tensor("sim", [Asz, Bsz], F32, kind="ExternalOutput")

    Exp = mybir.ActivationFunctionType.Exp
    mult = mybir.AluOpType.mult
    sub = mybir.AluOpType.subtract

    with tile.TileContext(nc) as tc:
        with (
            tc.tile_pool(name="const", bufs=1) as cpool,
            tc.tile_pool(name="ring", bufs=1, space="PSUM") as rpool,
            tc.tile_pool(name="x", bufs=6) as x_pool,
            tc.tile_pool(name="e", bufs=5) as e_pool,
            tc.tile_pool(name="w", bufs=6) as w_pool,
            tc.tile_pool(name="rc", bufs=5) as rc_pool,
            tc.tile_pool(name="d", bufs=4) as d_pool,
            tc.tile_pool(name="sq", bufs=4) as sq_pool,
            tc.tile_pool(name="sr", bufs=3) as sr_pool,
        ):
            wt_sb = cpool.tile([128, KC, 240], BF16, tag="wt")
            eye_sb = cpool.tile([128, 128], BF16, tag="eye")
            # kv/qv: [d(128 zero-padded), batch, {q|k, v}, n] d-major
            kvT = cpool.tile([128, Asz, 2, N], BF16, tag="kvT")
            qvT = cpool.tile([128, Bsz, 2, N], BF16, tag="qvT")
            # n-major v with ones col: [n-chunk rows, chunk, batch, 81]
            va_cat = cpool.tile([128, 2, Asz, 81], BF16, tag="va")
            vb_cat = cpool.tile([128, 2, Bsz, 81], BF16, tag="vb")
            negones = cpool.tile([128, 1], BF16, tag="negones")
            ring = rpool.tile([128, 8, 512], F32, tag="ring")

            # one-time init
            nc.sync.dma_start(wt_sb, wt.ap().rearrange("k p c -> p k c"))
            nc.sync.dma_start(eye_sb, eye.ap())
            nc.gpsimd.memset(kvT[:], 0.0)
            nc.gpsimd.memset(qvT[:], 0.0)
            nc.gpsimd.memset(negones[:], -1.0)
            nc.gpsimd.memset(va_cat[:, :, :, 80:81], 1.0)
            nc.gpsimd.memset(vb_cat[:, :, :, 80:81], 1.0)
            nc.vector.memset(ring[:], 0.0)

            def qkv_batch(src, idx, want, qkv_dst, v_cat, wq):
                """Project one batch: q|k and v into one psum bank, one ACT
                copy out (d-major); PE-transpose v into n-major chunks packed
                side by side in one bank, one ACT copy into v_cat."""
                xt = x_pool.tile([128, KC, N], BF16, tag="x")
                nc.sync.dma_start(xt, src[idx].rearrange("k p n -> p k n"))
                c0 = 0 if want == "b" else 80   # q for b, k for a
                for kc in range(KC):
                    nc.tensor.matmul(ring[0:80, wq, 0:N],
                                     wt_sb[:, kc, c0:c0 + 80],
                                     xt[:, kc, :], start=(kc == 0),
                                     stop=(kc == KC - 1))
                for kc in range(KC):
                    nc.tensor.matmul(ring[0:80, wq, N:2 * N],
                                     wt_sb[:, kc, 160:240],
                                     xt[:, kc, :], start=(kc == 0),
                                     stop=(kc == KC - 1))
                nc.scalar.copy(qkv_dst, ring[0:80, wq, 0:2 * N])
                # transpose the v half into [n, d] chunks (bf16 psum views
                # packed in the SAME bank after the q|v block), then one
                # copy into v_cat — the whole batch occupies one psum slot
                # outside the subgroup windows' reuse path
                t0 = ring[0:128, wq, 392:432].bitcast(BF16)
                nc.tensor.matmul(t0, qvT[0:80, idx, 1, 0:128] if want == "b"
                                 else kvT[0:80, idx, 1, 0:128],
                                 eye_sb[0:80, 0:80],
                                 start=True, stop=True, is_transpose=True)
                t1 = ring[0:68, wq, 432:472].bitcast(BF16)
                nc.tensor.matmul(t1, qvT[0:80, idx, 1, 128:N] if want == "b"
                                 else kvT[0:80, idx, 1, 128:N],
                                 eye_sb[0:80, 0:80],
                                 start=True, stop=True, is_transpose=True)
                tcat = ring[0:128, wq, 392:472].bitcast(BF16)
                nc.scalar.copy(
                    v_cat[0:128, :, idx, 0:80],
                    tcat.rearrange("p (c d) -> p c d", c=2))

            def mm2_sg(p1, ncx):
                """Deferred mm2 for one n-chunk: U'[n, 4x81] accumulated
                over m-chunks; reuses slot base+ncx after exp read it."""
                (base, a, b0, simrow, dma_rows), es = p1
                ncols = 128 if ncx == 0 else 68
                for p in range(4):
                    for mc in range(2):
                        mr = 128 if mc == 0 else 68
                        nc.tensor.matmul(
                            ring[0:ncols, base + ncx, 81 * p:81 * p + 81],
                            es[0:mr, 2 * mc + p // 2,
                               (p % 2) * 196 + 128 * ncx:
                               (p % 2) * 196 + 128 * ncx + ncols],
                            va_cat[0:mr, mc, a, 0:81],
                            start=(mc == 0), stop=(mc == 1),
                        )

            def early_tail(p, ncx, rec):
                """psum-consuming tail of one n-chunk: reciprocal of the
                denominators and the fused normalize (emitted right after
                that chunk's mm2 so it overlaps the other chunk's mm2)."""
                ub, a, b0, simrow, dma_rows = p
                ncols = 128 if ncx == 0 else 68
                u3 = ring[0:ncols, ub + ncx, 0:324].rearrange(
                    "p (k d) -> p k d", k=4)
                den = u3[:, :, 80:81].rearrange("p k d -> p (k d)")
                w_t = w_pool.tile([128, 4], F32, tag="w")
                nc.vector.reciprocal(w_t[0:ncols, :], den)
                wbc = bass.AP(w_t.tensor, w_t.offset,
                              [[4, ncols], [1, 4], [0, 81]])
                nc.vector.tensor_tensor(rec[0:ncols, ncx, :, :], u3,
                                        wbc, op=mult)

            def mid_tail(p, rec):
                """SBUF-only: subtract (DVE) and square (Pool)."""
                ub, a, b0, simrow, dma_rows = p
                d_t = d_pool.tile([128, 2, 4, 81], BF16, tag="d")
                nc.vector.tensor_tensor(
                    d_t, rec, vb_cat[:, :, b0:b0 + 4, :], op=sub)
                sq_t = sq_pool.tile([128, 2, 4, 81], BF16, tag="sq")
                nc.gpsimd.tensor_tensor(sq_t, d_t, d_t, op=mult)
                return sq_t

            def fin_tail(p, sq_t):
                """-sum over n (matmul into a spare slot of the other
                window), then sum over d; output DMA after the last
                subgroup of a 4-a group."""
                ub, a, b0, simrow, dma_rows = p
                sim_ps = ring[0:1, ub + 3, 0:324]
                nc.tensor.matmul(
                    sim_ps, negones[:, 0:1],
                    sq_t[0:128, 0, :, :].rearrange("p k d -> p (k d)"),
                    start=True, stop=False)
                nc.tensor.matmul(
                    sim_ps, negones[0:68, 0:1],
                    sq_t[0:68, 1, :, :].rearrange("p k d -> p (k d)"),
                    start=False, stop=True)
                nc.vector.reduce_sum(
                    out=simrow[0:1, a % 4, b0:b0 + 4],
                    in_=sim_ps.rearrange("p (k d) -> p k d", k=4),
                    axis=mybir.AxisListType.X,
                )
                if dma_rows is not None:
                    nc.sync.dma_start(simo[dma_rows[0]:dma_rows[1], :], simrow)

            # phase 1: the first 4 b batches up front; the rest are
            # interleaved into the early subgroup pipeline (subgroup sgb of
            # any a only needs b batches 4*sgb..4*sgb+3)
            for b in range(4):
                qkv_batch(fb, b, "b", qvT[0:80, b, :, :], vb_cat,
                          3 + 4 * (b % 2))

            pend1 = None  # awaiting early tail (deferred 1 subgroup)
            pend2 = None  # awaiting mid tail (deferred 2 subgroups)
            pend3 = None  # awaiting fin tail (deferred 3 subgroups)
            for a in range(Asz):
                # slots 0/1: the window the upcoming sg0 uses -- its previous
                # tenant (U' of the last even-window sg) was consumed one
                # subgroup ago, and windows alternate strictly per subgroup so
                # a deferred early_tail never trails its window's next writer
                qkv_batch(fa, a, "a", kvT[0:80, a, :, :], va_cat, 7)
                if a % 4 == 0:
                    simrow = sr_pool.tile([1, 4, Bsz], F32, tag="sr")

                for sgb in range(NSG):
                    b0 = 4 * sgb
                    # feed the remaining b batches during a=0: subgroup sgb
                    # needs b0..b0+3 of the NEXT subgroup ready one step
                    # ahead, so emit 4 per subgroup slot-alternating
                    if a == 0 and sgb < 3:
                        for bb in range(4 * sgb + 4, 4 * sgb + 8):
                            qkv_batch(fb, bb, "b", qvT[0:80, bb, :, :],
                                      vb_cat, 3 + 4 * (bb % 2))
                    base = 4 * ((a * NSG + sgb) % 2)
                    es = e_pool.tile([128, 4, 392], BF16, tag="e")
                    # 5-stage software pipeline over subgroups: mm2 + early
                    # tail at deferral 1, mid (sub+square) at 2, fin
                    # (n-reduce + d-reduce) at 3. mm1/exp are split per
                    # m-chunk and mm2(g-1) is emitted between the halves so
                    # the U'(g-1) -> mult(g-1) chain (which releases the
                    # next window) starts as early as possible while exp(g)
                    # still overlaps mm1(g).
                    for pp in range(2):
                        nc.tensor.matmul(
                            ring[0:128, base + pp, 0:392],
                            kvT[:, a, 0, 0:128],
                            qvT[:, b0 + 2 * pp:b0 + 2 * pp + 2, 0, :],
                            start=True, stop=True,
                        )
                    nc.scalar.activation(es[:, 0:2, :],
                                         ring[:, base:base + 2, 0:392], Exp)
                    nxt1 = None
                    if pend1 is not None:
                        rec = rc_pool.tile([128, 2, 4, 81], BF16, tag="rec")
                        mm2_sg(pend1, 0)
                        early_tail(pend1[0], 0, rec)
                        nxt1 = (pend1[0], rec)
                    for pp in range(2):
                        nc.tensor.matmul(
                            ring[0:68, base + 2 + pp, 0:392],
                            kvT[:, a, 0, 128:196],
                            qvT[:, b0 + 2 * pp:b0 + 2 * pp + 2, 0, :],
                            start=True, stop=True,
                        )
                    nc.scalar.activation(es[:, 2:4, :],
                                         ring[:, base + 2:base + 4, 0:392], Exp)
                    if pend1 is not None:
                        mm2_sg(pend1, 1)
                        early_tail(pend1[0], 1, nxt1[1])
                    nxt2 = (pend2[0], mid_tail(*pend2)) if pend2 is not None \
                        else None
                    if pend3 is not None:
                        fin_tail(*pend3)
                    pend3, pend2 = nxt2, nxt1
                    dma_rows = (a - 3, a + 1) if (a % 4 == 3 and
                                                  sgb == NSG - 1) else None
                    pend1 = ((base, a, b0, simrow, dma_rows), es)
            if pend1 is not None:
                rec = rc_pool.tile([128, 2, 4, 81], BF16, tag="rec")
                mm2_sg(pend1, 0)
                early_tail(pend1[0], 0, rec)
                mm2_sg(pend1, 1)
                early_tail(pend1[0], 1, rec)
                pend0 = (pend1[0], rec)
            else:
                pend0 = None
            if pend3 is not None:
                fin_tail(*pend3)
            if pend2 is not None:
                fin_tail(pend2[0], mid_tail(*pend2))
            if pend0 is not None:
                fin_tail(pend0[0], mid_tail(*pend0))

    return nc


def _split_multi_waits(nc):
    """This walrus build accepts at most one semaphore wait per instruction;
    Tile emits several (incl. its tail drain). Hoist extra waits onto
    single-wait engine NoOps inserted just before the instruction."""
    cnt = 0
    for f in nc.m.functions:
        for bb in f.blocks:
            insts = list(bb.instructions)
            out = []
            changed = False
            for inst in insts:
                si = getattr(inst, "sync_info", None)
                ws = list(si.on_wait) if (si is not None and si.on_wait) else []
                if len(ws) > 1:
                    changed = True
                    for w in ws[:-1]:
                        cnt += 1
                        out.append(mybir.InstNoOp(
                            name=f"WSPLIT-{cnt}",
                            engine=inst.engine,
                            ins=[], outs=[],
                            sync_info=mybir.SyncInfo(on_wait=[w], on_update=[]),
                        ))
                    si.on_wait = [ws[-1]]
                    inst.sync_info = si
                out.append(inst)
            if changed:
                bb.instructions = out
    return nc


def _get_program(Asz, Bsz):
    key = (Asz, Bsz)
    if key not in _PROGRAM_CACHE:
        _PROGRAM_CACHE[key] = _split_multi_waits(_build(Asz, Bsz))
    return _PROGRAM_CACHE[key]


def _prep_inputs(features_a, features_b, W_qkv, Asz, Bsz, ncores):
    """Host-side: cast to bf16, fold the 1/sqrt(D) scale into Wq, reshape."""
    fa = features_a.reshape(Asz, HID, N).astype(ml_dtypes.bfloat16)
    fa = fa.reshape(Asz, KC, 128, N)
    wt = W_qkv.T.copy().astype(np.float32)   # [640, 240]
    wt[:, 0:D] *= SCALE
    wt = wt.astype(ml_dtypes.bfloat16).reshape(KC, 128, 240)
    eye = np.eye(128, dtype=ml_dtypes.bfloat16)
    fbs = []
    for c in range(ncores):
        fb = features_b[c * Bsz:(c + 1) * Bsz].reshape(Bsz, HID, N)
        fb = fb.astype(ml_dtypes.bfloat16).reshape(Bsz, KC, 128, N)
        fbs.append(fb)
    return fa, fbs, wt, eye


def kernel(features_a, features_b, W_qkv):
    Asz = features_a.shape[0]
    Bfull = features_b.shape[0]
    ncores = NCORES
    Bsz = Bfull // ncores
    fa, fbs, wt, eye = _prep_inputs(
        np.asarray(features_a), np.asarray(features_b), np.asarray(W_qkv),
        Asz, Bsz, ncores,
    )
    nc = _get_program(Asz, Bsz)
    in_maps = [{"fa": fa, "fb": fbs[c], "wt": wt, "eye": eye}
               for c in range(ncores)]
    res = run_bass_kernel_spmd(nc, in_maps, core_ids=list(range(ncores)))
    out = np.concatenate([res.results[c]["sim"].T for c in range(ncores)], axis=0)
    return out.astype(np.float32)


# revision 37
# speedup vs baseline: 2.3101x; 1.0860x over previous
"""Trainium2 Bass kernel for nn_CLARM_56693568307877.

Computes, for feature sets A [64,640,14,14] and B [128,640,14,14] and a QKV
projection W [240,640]:
    q,k,v = split(x^T W^T); S = q_b k_a^T / sqrt(80); P = softmax(S)
    rec = P v_a;  sim[b,a] = -||v_b - rec||^2_F
Output [128, 64] fp32.

Sharding: data-parallel over the b batch (16 per core x 8 cores);
features_a / W replicated. Device math in bf16 with fp32 accumulation
(max rel err ~3.4e-4 vs the fp32 reference).

Design (per-core B=16, A=64, N=196 tokens, D=80), n-major attention tail:
  phase 1 (one psum slot per batch, outside the subgroup windows):
    QKV on PE (W^T stationary, q|k and v packed into one bank), one ACT
    copy out d-major; v is PE-transposed (identity matmul, bf16 psum
    views packed in the same bank) into n-major chunk tiles
    va_cat/vb_cat [128/68 rows, chunk, batch, 81] with ones at col 80.
  phase 2, per (a, 4-b subgroup) on two alternating 4-slot psum windows:
    mm1  S^T chunks [mchunk, 392] for 4 pairs -> the window's 4 slots
    exp  one ACT instruction per m-chunk -> es bf16 SBUF
    mm2  U'[n, 4x81] = es^T @ [v_a|1] per n-chunk (den lands in col 80)
    tail w = 1/den (DVE, free-4); rec = U'*w via a stride-0 broadcast
         multiply (doubles as the psum egress, DVE); D = rec - vbn
         (DVE, 2x); D^2 on Pool; -sum over n via a (-1)-stationary
         matmul into a spare slot (row 0) of the own window; sum over d
         with one partition-1 reduce straight into the output row.
  The subgroup stages run as a 5-deep software pipeline (mm1+exp | mm2 +
  reciprocal/normalize, per n-chunk, sandwiched between the mm1 halves |
  subtract+square | n-reduce+d-reduce), so every engine's in-order queue
  only ever holds work whose inputs are already available, and the psum
  window of subgroup g is recycled by mm1(g+2) right after normalize(g).

Note: this walrus build accepts at most one semaphore wait per
instruction; _split_multi_waits hoists extras onto NoOps.
"""

import numpy as np
import ml_dtypes

import concourse.bass as bass
import concourse.tile as tile
from concourse import mybir
from concourse.bass_utils import run_bass_kernel_spmd

BF16 = mybir.dt.bfloat16
F32 = mybir.dt.float32

NCORES = 8
A_FULL = 64
B_FULL = 128
HID = 640
KC = HID // 128  # 5
N = 196          # tokens (14*14)
D = 80           # inner dim
MPAD = 256       # m padded to 2*128 so mm1 writes full psum rows
SCALE = 1.0 / np.sqrt(D)

_PROGRAM_CACHE = {}


def _build(Asz, Bsz):
    """Emit the Bass program for one core handling Bsz b's x Asz a's."""
    assert Bsz % 4 == 0 and Asz % 4 == 0
    NSG = Bsz // 4            # 4-b subgroups per a

    nc = bass.Bass("TRN2", debug=False)
    fa = nc.dram_tensor("fa", [Asz, KC, 128, N], BF16, kind="ExternalInput")
    fb = nc.dram_tensor("fb", [Bsz, KC, 128, N], BF16, kind="ExternalInput")
    wt = nc.dram_tensor("wt", [KC, 128, 240], BF16, kind="ExternalInput")
    eye = nc.dram_tensor("eye", [128, 128], BF16, kind="ExternalInput")
    simo = nc.dram_tensor("sim", [Asz, Bsz], F32, kind="ExternalOutput")

    Exp = mybir.ActivationFunctionType.Exp
    mult = mybir.AluOpType.mult
    sub = mybir.AluOpType.subtract

    with tile.TileContext(nc) as tc:
        with (
            tc.tile_pool(name="const", bufs=1) as cpool,
            tc.tile_pool(name="ring", bufs=1, space="PSUM") as rpool,
            tc.tile_pool(name="x", bufs=6) as x_pool,
            tc.tile_pool(name="e", bufs=5) as e_pool,
            tc.tile_pool(name="w", bufs=6) as w_pool,
            tc.tile_pool(name="rc", bufs=5) as rc_pool,
            tc.tile_pool(name="d", bufs=4) as d_pool,
            tc.tile_pool(name="sq", bufs=4) as sq_pool,
            tc.tile_pool(name="sr", bufs=3) as sr_pool,
        ):
            wt_sb = cpool.tile([128, KC, 240], BF16, tag="wt")
            eye_sb = cpool.tile([128, 128], BF16, tag="eye")
            # kv/qv: [d(128 zero-padded), batch, {q|k, v}, n] d-major
            kvT = cpool.tile([128, Asz, 2, N], BF16, tag="kvT")
            qvT = cpool.tile([128, Bsz, 2, N], BF16, tag="qvT")
            # n-major v with ones col: [n-chunk rows, chunk, batch, 81]
            va_cat = cpool.tile([128, 2, Asz, 81], BF16, tag="va")
            vb_cat = cpool.tile([128, 2, Bsz, 81], BF16, tag="vb")
            negones = cpool.tile([128, 1], BF16, tag="negones")
            ring = rpool.tile([128, 8, 512], F32, tag="ring")

            # one-time init
            nc.sync.dma_start(wt_sb, wt.ap().rearrange("k p c -> p k c"))
            nc.sync.dma_start(eye_sb, eye.ap())
            nc.gpsimd.memset(kvT[:], 0.0)
            nc.gpsimd.memset(qvT[:], 0.0)
            nc.gpsimd.memset(negones[:], -1.0)
            nc.gpsimd.memset(va_cat[:, :, :, 80:81], 1.0)
            nc.gpsimd.memset(vb_cat[:, :, :, 80:81], 1.0)
            nc.vector.memset(ring[:], 0.0)

            def qkv_mm(src, idx, want, wq):
                """Project one batch: q|k and v into one psum bank."""
                xt = x_pool.tile([128, KC, N], BF16, tag="x")
                nc.sync.dma_start(xt, src[idx].rearrange("k p n -> p k n"))
                c0 = 0 if want == "b" else 80   # q for b, k for a
                for kc in range(KC):
                    nc.tensor.matmul(ring[0:80, wq, 0:N],
                                     wt_sb[:, kc, c0:c0 + 80],
                                     xt[:, kc, :], start=(kc == 0),
                                     stop=(kc == KC - 1))
                for kc in range(KC):
                    nc.tensor.matmul(ring[0:80, wq, N:2 * N],
                                     wt_sb[:, kc, 160:240],
                                     xt[:, kc, :], start=(kc == 0),
                                     stop=(kc == KC - 1))

            def qkv_batch(src, idx, want, qkv_dst, v_cat, wq):
                """Copy the projections out (d-major), then PE-transpose the
                v half into n-major chunks and copy into v_cat."""
                nc.scalar.copy(qkv_dst, ring[0:80, wq, 0:2 * N])
                # transpose the v half into [n, d] chunks (bf16 psum views
                # packed in the SAME bank after the q|v block), then one
                # copy into v_cat — the whole batch occupies one psum slot
                # outside the subgroup windows' reuse path
                t0 = ring[0:128, wq, 392:432].bitcast(BF16)
                nc.tensor.matmul(t0, qvT[0:80, idx, 1, 0:128] if want == "b"
                                 else kvT[0:80, idx, 1, 0:128],
                                 eye_sb[0:80, 0:80],
                                 start=True, stop=True, is_transpose=True)
                t1 = ring[0:68, wq, 432:472].bitcast(BF16)
                nc.tensor.matmul(t1, qvT[0:80, idx, 1, 128:N] if want == "b"
                                 else kvT[0:80, idx, 1, 128:N],
                                 eye_sb[0:80, 0:80],
                                 start=True, stop=True, is_transpose=True)
                tcat = ring[0:128, wq, 392:472].bitcast(BF16)
                nc.scalar.copy(
                    v_cat[0:128, :, idx, 0:80],
                    tcat.rearrange("p (c d) -> p c d", c=2))

            def mm2_sg(p1, ncx):
                """Deferred mm2 for one n-chunk: U'[n, 4x81] accumulated
                over m-chunks; reuses slot base+ncx after exp read it."""
                (base, a, b0, simrow, dma_rows), es = p1
                ncols = 128 if ncx == 0 else 68
                for p in range(4):
                    for mc in range(2):
                        mr = 128 if mc == 0 else 68
                        nc.tensor.matmul(
                            ring[0:ncols, base + ncx, 81 * p:81 * p + 81],
                            es[0:mr, 2 * mc + p // 2,
                               (p % 2) * 196 + 128 * ncx:
                               (p % 2) * 196 + 128 * ncx + ncols],
                            va_cat[0:mr, mc, a, 0:81],
                            start=(mc == 0), stop=(mc == 1),
                        )

            def early_tail(p, ncx, rec):
                """psum-consuming tail of one n-chunk: reciprocal of the
                denominators and the fused normalize (emitted right after
                that chunk's mm2 so it overlaps the other chunk's mm2)."""
                ub, a, b0, simrow, dma_rows = p
                ncols = 128 if ncx == 0 else 68
                u3 = ring[0:ncols, ub + ncx, 0:324].rearrange(
                    "p (k d) -> p k d", k=4)
                den = u3[:, :, 80:81].rearrange("p k d -> p (k d)")
                w_t = w_pool.tile([128, 4], F32, tag="w")
                nc.vector.reciprocal(w_t[0:ncols, :], den)
                wbc = bass.AP(w_t.tensor, w_t.offset,
                              [[4, ncols], [1, 4], [0, 81]])
                nc.vector.tensor_tensor(rec[0:ncols, ncx, :, :], u3,
                                        wbc, op=mult)

            def mid_tail(p, rec):
                """SBUF-only: subtract (DVE) and square (Pool)."""
                ub, a, b0, simrow, dma_rows = p
                d_t = d_pool.tile([128, 2, 4, 81], BF16, tag="d")
                nc.vector.tensor_tensor(
                    d_t, rec, vb_cat[:, :, b0:b0 + 4, :], op=sub)
                sq_t = sq_pool.tile([128, 2, 4, 81], BF16, tag="sq")
                nc.gpsimd.tensor_tensor(sq_t, d_t, d_t, op=mult)
                return sq_t

            def fin_tail(p, sq_t):
                """-sum over n (matmul into a spare slot of the other
                window), then sum over d; output DMA after the last
                subgroup of a 4-a group."""
                ub, a, b0, simrow, dma_rows = p
                # slot 3 row 0 for every subgroup's n-reduce: its S^T
                # tenant's exp always precedes the nred in program order,
                # and slot 7 stays exclusively qkv's
                sim_ps = ring[0:1, 3, 0:324]
                nc.tensor.matmul(
                    sim_ps, negones[:, 0:1],
                    sq_t[0:128, 0, :, :].rearrange("p k d -> p (k d)"),
                    start=True, stop=False)
                nc.tensor.matmul(
                    sim_ps, negones[0:68, 0:1],
                    sq_t[0:68, 1, :, :].rearrange("p k d -> p (k d)"),
                    start=False, stop=True)
                nc.vector.reduce_sum(
                    out=simrow[0:1, a % 4, b0:b0 + 4],
                    in_=sim_ps.rearrange("p (k d) -> p k d", k=4),
                    axis=mybir.AxisListType.X,
                )
                if dma_rows is not None:
                    nc.sync.dma_start(simo[dma_rows[0]:dma_rows[1], :], simrow)

            # phase 1: the first 4 b batches up front; the rest are
            # interleaved into the early subgroup pipeline (subgroup sgb of
            # any a only needs b batches 4*sgb..4*sgb+3)
            for b in range(4):
                qkv_mm(fb, b, "b", 3 + 4 * (b % 2))
                qkv_batch(fb, b, "b", qvT[0:80, b, :, :], vb_cat,
                          3 + 4 * (b % 2))

            pend1 = None  # awaiting early tail (deferred 1 subgroup)
            pend2 = None  # awaiting mid tail (deferred 2 subgroups)
            pend3 = None  # awaiting fin tail (deferred 3 subgroups)
            # first a's projections up front; later ones are prefetched
            # two subgroups before their a-boundary (kv-copy must precede
            # mm1 of the next a's first subgroup)
            qkv_mm(fa, 0, "a", 7)
            qkv_batch(fa, 0, "a", kvT[0:80, 0, :, :], va_cat, 7)
            for a in range(Asz):
                if a % 4 == 0:
                    simrow = sr_pool.tile([1, 4, Bsz], F32, tag="sr")

                for sgb in range(NSG):
                    b0 = 4 * sgb
                    # feed the remaining b batches during a=0: subgroup sgb
                    # needs b0..b0+3 of the NEXT subgroup ready one step
                    # ahead, so emit 4 per subgroup slot-alternating
                    if a == 0 and sgb < 3:
                        for bb in range(4 * sgb + 4, 4 * sgb + 8):
                            qkv_mm(fb, bb, "b", 3 + 4 * (bb % 2))
                            qkv_batch(fb, bb, "b", qvT[0:80, bb, :, :],
                                      vb_cat, 3 + 4 * (bb % 2))
                    if sgb == 2 and a + 1 < Asz:
                        qkv_mm(fa, a + 1, "a", 7)
                    if sgb == 3 and a + 1 < Asz:
                        qkv_batch(fa, a + 1, "a", kvT[0:80, a + 1, :, :],
                                  va_cat, 7)
                    base = 4 * ((a * NSG + sgb) % 2)
                    es = e_pool.tile([128, 4, 392], BF16, tag="e")
                    # 5-stage software pipeline over subgroups: mm2 + early
                    # tail at deferral 1, mid (sub+square) at 2, fin
                    # (n-reduce + d-reduce) at 3. mm1/exp are split per
                    # m-chunk and mm2(g-1) is emitted between the halves so
                    # the U'(g-1) -> mult(g-1) chain (which releases the
                    # next window) starts as early as possible while exp(g)
                    # still overlaps mm1(g).
                    for pp in range(2):
                        nc.tensor.matmul(
                            ring[0:128, base + pp, 0:392],
                            kvT[:, a, 0, 0:128],
                            qvT[:, b0 + 2 * pp:b0 + 2 * pp + 2, 0, :],
                            start=True, stop=True,
                        )
                    nc.scalar.activation(es[:, 0:2, :],
                                         ring[:, base:base + 2, 0:392], Exp)
                    nxt1 = None
                    if pend1 is not None:
                        rec = rc_pool.tile([128, 2, 4, 81], BF16, tag="rec")
                        mm2_sg(pend1, 0)
                        early_tail(pend1[0], 0, rec)
                        nxt1 = (pend1[0], rec)
                    for pp in range(2):
                        nc.tensor.matmul(
                            ring[0:68, base + 2 + pp, 0:392],
                            kvT[:, a, 0, 128:196],
                            qvT[:, b0 + 2 * pp:b0 + 2 * pp + 2, 0, :],
                            start=True, stop=True,
                        )
                    nc.scalar.activation(es[:, 2:4, :],
                                         ring[:, base + 2:base + 4, 0:392], Exp)
                    if pend1 is not None:
                        mm2_sg(pend1, 1)
                        early_tail(pend1[0], 1, nxt1[1])
                    nxt2 = (pend2[0], mid_tail(*pend2)) if pend2 is not None \
                        else None
                    if pend3 is not None:
                        fin_tail(*pend3)
                    pend3, pend2 = nxt2, nxt1
                    dma_rows = (a - 3, a + 1) if (a % 4 == 3 and
                                                  sgb == NSG - 1) else None
                    pend1 = ((base, a, b0, simrow, dma_rows), es)
            if pend1 is not None:
                rec = rc_pool.tile([128, 2, 4, 81], BF16, tag="rec")
                mm2_sg(pend1, 0)
                early_tail(pend1[0], 0, rec)
                mm2_sg(pend1, 1)
                early_tail(pend1[0], 1, rec)
                pend0 = (pend1[0], rec)
            else:
                pend0 = None
            if pend3 is not None:
                fin_tail(*pend3)
            if pend2 is not None:
                fin_tail(pend2[0], mid_tail(*pend2))
            if pend0 is not None:
                fin_tail(pend0[0], mid_tail(*pend0))

    return nc


def _split_multi_waits(nc):
    """This walrus build accepts at most one semaphore wait per instruction;
    Tile emits several (incl. its tail drain). Hoist extra waits onto
    single-wait engine NoOps inserted just before the instruction."""
    cnt = 0
    for f in nc.m.functions:
        for bb in f.blocks:
            insts = list(bb.instructions)
            out = []
            changed = False
            for inst in insts:
                si = getattr(inst, "sync_info", None)
                ws = list(si.on_wait) if (si is not None and si.on_wait) else []
                if len(ws) > 1:
                    changed = True
                    for w in ws[:-1]:
                        cnt += 1
                        out.append(mybir.InstNoOp(
                            name=f"WSPLIT-{cnt}",
                            engine=inst.engine,
                            ins=[], outs=[],
                            sync_info=mybir.SyncInfo(on_wait=[w], on_update=[]),
                        ))
                    si.on_wait = [ws[-1]]
                    inst.sync_info = si
                out.append(inst)
            if changed:
                bb.instructions = out
    return nc


def _get_program(Asz, Bsz):
    key = (Asz, Bsz)
    if key not in _PROGRAM_CACHE:
        _PROGRAM_CACHE[key] = _split_multi_waits(_build(Asz, Bsz))
    return _PROGRAM_CACHE[key]


def _prep_inputs(features_a, features_b, W_qkv, Asz, Bsz, ncores):
    """Host-side: cast to bf16, fold the 1/sqrt(D) scale into Wq, reshape."""
    fa = features_a.reshape(Asz, HID, N).astype(ml_dtypes.bfloat16)
    fa = fa.reshape(Asz, KC, 128, N)
    wt = W_qkv.T.copy().astype(np.float32)   # [640, 240]
    wt[:, 0:D] *= SCALE
    wt = wt.astype(ml_dtypes.bfloat16).reshape(KC, 128, 240)
    eye = np.eye(128, dtype=ml_dtypes.bfloat16)
    fbs = []
    for c in range(ncores):
        fb = features_b[c * Bsz:(c + 1) * Bsz].reshape(Bsz, HID, N)
        fb = fb.astype(ml_dtypes.bfloat16).reshape(Bsz, KC, 128, N)
        fbs.append(fb)
    return fa, fbs, wt, eye


def kernel(features_a, features_b, W_qkv):
    Asz = features_a.shape[0]
    Bfull = features_b.shape[0]
    ncores = NCORES
    Bsz = Bfull // ncores
    fa, fbs, wt, eye = _prep_inputs(
        np.asarray(features_a), np.asarray(features_b), np.asarray(W_qkv),
        Asz, Bsz, ncores,
    )
    nc = _get_program(Asz, Bsz)
    in_maps = [{"fa": fa, "fb": fbs[c], "wt": wt, "eye": eye}
               for c in range(ncores)]
    res = run_bass_kernel_spmd(nc, in_maps, core_ids=list(range(ncores)))
    out = np.concatenate([res.results[c]["sim"].T for c in range(ncores)], axis=0)
    return out.astype(np.float32)


# revision 41
# speedup vs baseline: 2.3911x; 1.0350x over previous
"""Trainium2 Bass kernel for nn_CLARM_56693568307877.

Computes, for feature sets A [64,640,14,14] and B [128,640,14,14] and a QKV
projection W [240,640]:
    q,k,v = split(x^T W^T); S = q_b k_a^T / sqrt(80); P = softmax(S)
    rec = P v_a;  sim[b,a] = -||v_b - rec||^2_F
Output [128, 64] fp32.

Sharding: data-parallel over the b batch (16 per core x 8 cores);
features_a / W replicated. Device math in bf16 with fp32 accumulation
(max rel err ~3.4e-4 vs the fp32 reference).

Design (per-core B=16, A=64, N=196 tokens, D=80), n-major attention tail:
  phase 1 (one psum slot per batch, outside the subgroup windows):
    QKV on PE (W^T stationary, q|k and v packed into one bank), one ACT
    copy out d-major; v is PE-transposed (identity matmul, bf16 psum
    views packed in the same bank) into n-major chunk tiles
    va_cat/vb_cat [128/68 rows, chunk, batch, 81] with ones at col 80.
  phase 2, per (a, 4-b subgroup) on two alternating 4-slot psum windows:
    mm1  S^T chunks [mchunk, 392] for 4 pairs -> the window's 4 slots
    exp  one ACT instruction per m-chunk -> es bf16 SBUF
    mm2  U'[n, 4x81] = es^T @ [v_a|1] per n-chunk (den lands in col 80)
    tail w = 1/den (DVE, free-4); rec = U'*w via a stride-0 broadcast
         multiply (doubles as the psum egress, DVE); D = rec - vbn
         (DVE, 2x); D^2 on Pool; -sum over n via a (-1)-stationary
         matmul into a spare slot (row 0) of the own window; sum over d
         with one partition-1 reduce straight into the output row.
  The subgroup stages run as a 5-deep software pipeline (mm1+exp | mm2 +
  reciprocal/normalize, per n-chunk, sandwiched between the mm1 halves |
  subtract+square | n-reduce+d-reduce), so every engine's in-order queue
  only ever holds work whose inputs are already available, and the psum
  window of subgroup g is recycled by mm1(g+2) right after normalize(g).

Note: this walrus build accepts at most one semaphore wait per
instruction; _split_multi_waits hoists extras onto NoOps.
"""

import numpy as np
import ml_dtypes

import concourse.bass as bass
import concourse.tile as tile
from concourse import mybir
from concourse.bass_utils import run_bass_kernel_spmd

BF16 = mybir.dt.bfloat16
F32 = mybir.dt.float32

NCORES = 8
A_FULL = 64
B_FULL = 128
HID = 640
KC = HID // 128  # 5
N = 196          # tokens (14*14)
D = 80           # inner dim
MPAD = 256       # m padded to 2*128 so mm1 writes full psum rows
SCALE = 1.0 / np.sqrt(D)

_PROGRAM_CACHE = {}


def _build(Asz, Bsz):
    """Emit the Bass program for one core handling Bsz b's x Asz a's."""
    assert Bsz % 4 == 0 and Asz % 4 == 0
    NSG = Bsz // 4            # 4-b subgroups per a

    nc = bass.Bass("TRN2", debug=False)
    fa = nc.dram_tensor("fa", [Asz, KC, 128, N], BF16, kind="ExternalInput")
    fb = nc.dram_tensor("fb", [Bsz, KC, 128, N], BF16, kind="ExternalInput")
    wt = nc.dram_tensor("wt", [KC, 128, 240], BF16, kind="ExternalInput")
    eye = nc.dram_tensor("eye", [128, 128], BF16, kind="ExternalInput")
    simo = nc.dram_tensor("sim", [Asz, Bsz], F32, kind="ExternalOutput")

    Exp = mybir.ActivationFunctionType.Exp
    mult = mybir.AluOpType.mult
    sub = mybir.AluOpType.subtract

    with tile.TileContext(nc) as tc:
        with (
            tc.tile_pool(name="const", bufs=1) as cpool,
            tc.tile_pool(name="ring", bufs=1, space="PSUM") as rpool,
            tc.tile_pool(name="x", bufs=6) as x_pool,
            tc.tile_pool(name="e", bufs=5) as e_pool,
            tc.tile_pool(name="w", bufs=6) as w_pool,
            tc.tile_pool(name="rc", bufs=5) as rc_pool,
            tc.tile_pool(name="d", bufs=4) as d_pool,
            tc.tile_pool(name="sq", bufs=4) as sq_pool,
            tc.tile_pool(name="sr", bufs=3) as sr_pool,
        ):
            wt_sb = cpool.tile([128, KC, 240], BF16, tag="wt")
            eye_sb = cpool.tile([128, 128], BF16, tag="eye")
            # kv/qv: [d(128 zero-padded), batch, {q|k, v}, n] d-major
            kvT = cpool.tile([128, Asz, 2, N], BF16, tag="kvT")
            qvT = cpool.tile([128, Bsz, 2, N], BF16, tag="qvT")
            # n-major v with ones col: [n-chunk rows, chunk, batch, 81]
            va_cat = cpool.tile([128, 2, Asz, 81], BF16, tag="va")
            vb_cat = cpool.tile([128, 2, Bsz, 81], BF16, tag="vb")
            negones = cpool.tile([128, 1], BF16, tag="negones")
            ring = rpool.tile([128, 8, 512], F32, tag="ring")

            # one-time init
            nc.sync.dma_start(wt_sb, wt.ap().rearrange("k p c -> p k c"))
            nc.sync.dma_start(eye_sb, eye.ap())
            nc.gpsimd.memset(kvT[:], 0.0)
            nc.gpsimd.memset(qvT[:], 0.0)
            nc.gpsimd.memset(negones[:], -1.0)
            nc.gpsimd.memset(va_cat[:, :, :, 80:81], 1.0)
            nc.gpsimd.memset(vb_cat[:, :, :, 80:81], 1.0)
            nc.vector.memset(ring[:], 0.0)

            def qkv_mm(src, idx, want, wq):
                """Project one batch: q|k and v into one psum bank."""
                xt = x_pool.tile([128, KC, N], BF16, tag="x")
                nc.sync.dma_start(xt, src[idx].rearrange("k p n -> p k n"))
                c0 = 0 if want == "b" else 80   # q for b, k for a
                for kc in range(KC):
                    nc.tensor.matmul(ring[0:80, wq, 0:N],
                                     wt_sb[:, kc, c0:c0 + 80],
                                     xt[:, kc, :], start=(kc == 0),
                                     stop=(kc == KC - 1))
                for kc in range(KC):
                    nc.tensor.matmul(ring[0:80, wq, N:2 * N],
                                     wt_sb[:, kc, 160:240],
                                     xt[:, kc, :], start=(kc == 0),
                                     stop=(kc == KC - 1))

            def qkv_batch(src, idx, want, qkv_dst, v_cat, wq):
                """Copy the projections out (d-major), then PE-transpose the
                v half into n-major chunks and copy into v_cat."""
                nc.scalar.copy(qkv_dst, ring[0:80, wq, 0:2 * N])
                # transpose the v half into [n, d] chunks (bf16 psum views
                # packed in the SAME bank after the q|v block), then one
                # copy into v_cat — the whole batch occupies one psum slot
                # outside the subgroup windows' reuse path
                t0 = ring[0:128, wq, 392:432].bitcast(BF16)
                nc.tensor.matmul(t0, qvT[0:80, idx, 1, 0:128] if want == "b"
                                 else kvT[0:80, idx, 1, 0:128],
                                 eye_sb[0:80, 0:80],
                                 start=True, stop=True, is_transpose=True)
                t1 = ring[0:68, wq, 432:472].bitcast(BF16)
                nc.tensor.matmul(t1, qvT[0:80, idx, 1, 128:N] if want == "b"
                                 else kvT[0:80, idx, 1, 128:N],
                                 eye_sb[0:80, 0:80],
                                 start=True, stop=True, is_transpose=True)
                tcat = ring[0:128, wq, 392:472].bitcast(BF16)
                nc.scalar.copy(
                    v_cat[0:128, :, idx, 0:80],
                    tcat.rearrange("p (c d) -> p c d", c=2))

            def mm2_sg(p1, ncx):
                """Deferred mm2 for one n-chunk: U'[n, 4x81] accumulated
                over m-chunks; reuses slot base+ncx after exp read it."""
                (base, a, b0, simrow, dma_rows), es = p1
                ncols = 128 if ncx == 0 else 68
                for p in range(4):
                    for mc in range(2):
                        mr = 128 if mc == 0 else 68
                        nc.tensor.matmul(
                            ring[0:ncols, base + ncx, 81 * p:81 * p + 81],
                            es[0:mr, 2 * mc + p // 2,
                               (p % 2) * 196 + 128 * ncx:
                               (p % 2) * 196 + 128 * ncx + ncols],
                            va_cat[0:mr, mc, a, 0:81],
                            start=(mc == 0), stop=(mc == 1),
                        )

            def tail_recip(p, ncx):
                """reciprocal of one n-chunk's denominators (psum reader)."""
                ub, a, b0, simrow, dma_rows = p
                ncols = 128 if ncx == 0 else 68
                u3 = ring[0:ncols, ub + ncx, 0:324].rearrange(
                    "p (k d) -> p k d", k=4)
                den = u3[:, :, 80:81].rearrange("p k d -> p (k d)")
                w_t = w_pool.tile([128, 4], F32, tag="w")
                nc.vector.reciprocal(w_t[0:ncols, :], den)
                return w_t

            def tail_mult(p, ncx, w_t, rec):
                """fused normalize of one n-chunk (doubles as psum egress);
                emitted a few DVE ops after its reciprocal so the access
                ack of the reciprocal is hidden by independent work."""
                ub, a, b0, simrow, dma_rows = p
                ncols = 128 if ncx == 0 else 68
                u3 = ring[0:ncols, ub + ncx, 0:324].rearrange(
                    "p (k d) -> p k d", k=4)
                wbc = bass.AP(w_t.tensor, w_t.offset,
                              [[4, ncols], [1, 4], [0, 81]])
                nc.vector.tensor_tensor(rec[0:ncols, ncx, :, :], u3,
                                        wbc, op=mult)

            def mid_sub(p, rec):
                """SBUF-only subtract (DVE)."""
                ub, a, b0, simrow, dma_rows = p
                d_t = d_pool.tile([128, 2, 4, 81], BF16, tag="d")
                nc.vector.tensor_tensor(
                    d_t, rec, vb_cat[:, :, b0:b0 + 4, :], op=sub)
                return d_t

            def mid_sq(d_t):
                """SBUF-only square (Pool)."""
                sq_t = sq_pool.tile([128, 2, 4, 81], BF16, tag="sq")
                nc.gpsimd.tensor_tensor(sq_t, d_t, d_t, op=mult)
                return sq_t

            def fin_tail(p, sq_t):
                """-sum over n (matmul into a spare slot of the other
                window), then sum over d; output DMA after the last
                subgroup of a 4-a group."""
                ub, a, b0, simrow, dma_rows = p
                # slot 3 row 0 for every subgroup's n-reduce: its S^T
                # tenant's exp always precedes the nred in program order,
                # and slot 7 stays exclusively qkv's
                sim_ps = ring[0:1, 3, 0:324]
                nc.tensor.matmul(
                    sim_ps, negones[:, 0:1],
                    sq_t[0:128, 0, :, :].rearrange("p k d -> p (k d)"),
                    start=True, stop=False)
                nc.tensor.matmul(
                    sim_ps, negones[0:68, 0:1],
                    sq_t[0:68, 1, :, :].rearrange("p k d -> p (k d)"),
                    start=False, stop=True)
                nc.vector.reduce_sum(
                    out=simrow[0:1, a % 4, b0:b0 + 4],
                    in_=sim_ps.rearrange("p (k d) -> p k d", k=4),
                    axis=mybir.AxisListType.X,
                )
                if dma_rows is not None:
                    nc.sync.dma_start(simo[dma_rows[0]:dma_rows[1], :], simrow)

            # phase 1: the first 4 b batches up front; the rest are
            # interleaved into the early subgroup pipeline (subgroup sgb of
            # any a only needs b batches 4*sgb..4*sgb+3)
            for b in range(4):
                qkv_mm(fb, b, "b", 3 + 4 * (b % 2))
                qkv_batch(fb, b, "b", qvT[0:80, b, :, :], vb_cat,
                          3 + 4 * (b % 2))

            pend1 = None  # awaiting early tail (deferred 1 subgroup)
            pend2 = None  # awaiting mid tail (deferred 2 subgroups)
            pend3 = None  # awaiting fin tail (deferred 3 subgroups)
            # first a's projections up front; later ones are prefetched
            # two subgroups before their a-boundary (kv-copy must precede
            # mm1 of the next a's first subgroup)
            qkv_mm(fa, 0, "a", 7)
            qkv_batch(fa, 0, "a", kvT[0:80, 0, :, :], va_cat, 7)
            for a in range(Asz):
                if a % 4 == 0:
                    simrow = sr_pool.tile([1, 4, Bsz], F32, tag="sr")

                for sgb in range(NSG):
                    b0 = 4 * sgb
                    # feed the remaining b batches during a=0: subgroup sgb
                    # needs b0..b0+3 of the NEXT subgroup ready one step
                    # ahead, so emit 4 per subgroup slot-alternating
                    if a == 0 and sgb < 3:
                        for bb in range(4 * sgb + 4, 4 * sgb + 8):
                            qkv_mm(fb, bb, "b", 3 + 4 * (bb % 2))
                            qkv_batch(fb, bb, "b", qvT[0:80, bb, :, :],
                                      vb_cat, 3 + 4 * (bb % 2))
                    if sgb == 2 and a + 1 < Asz:
                        qkv_mm(fa, a + 1, "a", 7)
                    if sgb == 3 and a + 1 < Asz:
                        qkv_batch(fa, a + 1, "a", kvT[0:80, a + 1, :, :],
                                  va_cat, 7)
                    base = 4 * ((a * NSG + sgb) % 2)
                    es = e_pool.tile([128, 4, 392], BF16, tag="e")
                    # 5-stage software pipeline over subgroups: mm2 + early
                    # tail at deferral 1, mid (sub+square) at 2, fin
                    # (n-reduce + d-reduce) at 3. mm1/exp are split per
                    # m-chunk and mm2(g-1) is emitted between the halves so
                    # the U'(g-1) -> mult(g-1) chain (which releases the
                    # next window) starts as early as possible while exp(g)
                    # still overlaps mm1(g).
                    for pp in range(2):
                        nc.tensor.matmul(
                            ring[0:128, base + pp, 0:392],
                            kvT[:, a, 0, 0:128],
                            qvT[:, b0 + 2 * pp:b0 + 2 * pp + 2, 0, :],
                            start=True, stop=True,
                        )
                    nc.scalar.activation(es[:, 0:2, :],
                                         ring[:, base:base + 2, 0:392], Exp)
                    nxt1 = None
                    if pend1 is not None:
                        rec = rc_pool.tile([128, 2, 4, 81], BF16, tag="rec")
                        mm2_sg(pend1, 0)
                        w0 = tail_recip(pend1[0], 0)
                        nxt1 = (pend1[0], rec)
                    d_t = mid_sub(*pend2) if pend2 is not None else None
                    if pend1 is not None:
                        tail_mult(pend1[0], 0, w0, nxt1[1])
                    nxt2 = (pend2[0], mid_sq(d_t)) if pend2 is not None \
                        else None
                    for pp in range(2):
                        nc.tensor.matmul(
                            ring[0:68, base + 2 + pp, 0:392],
                            kvT[:, a, 0, 128:196],
                            qvT[:, b0 + 2 * pp:b0 + 2 * pp + 2, 0, :],
                            start=True, stop=True,
                        )
                    nc.scalar.activation(es[:, 2:4, :],
                                         ring[:, base + 2:base + 4, 0:392], Exp)
                    if pend1 is not None:
                        mm2_sg(pend1, 1)
                        w1 = tail_recip(pend1[0], 1)
                    if pend3 is not None:
                        fin_tail(*pend3)
                    if pend1 is not None:
                        tail_mult(pend1[0], 1, w1, nxt1[1])
                    pend3, pend2 = nxt2, nxt1
                    dma_rows = (a - 3, a + 1) if (a % 4 == 3 and
                                                  sgb == NSG - 1) else None
                    pend1 = ((base, a, b0, simrow, dma_rows), es)
            if pend1 is not None:
                rec = rc_pool.tile([128, 2, 4, 81], BF16, tag="rec")
                mm2_sg(pend1, 0)
                tail_mult(pend1[0], 0, tail_recip(pend1[0], 0), rec)
                mm2_sg(pend1, 1)
                tail_mult(pend1[0], 1, tail_recip(pend1[0], 1), rec)
                pend0 = (pend1[0], rec)
            else:
                pend0 = None
            if pend3 is not None:
                fin_tail(*pend3)
            if pend2 is not None:
                fin_tail(pend2[0], mid_sq(mid_sub(*pend2)))
            if pend0 is not None:
                fin_tail(pend0[0], mid_sq(mid_sub(*pend0)))

    return nc


def _split_multi_waits(nc):
    """This walrus build accepts at most one semaphore wait per instruction;
    Tile emits several (incl. its tail drain). Hoist extra waits onto
    single-wait engine NoOps inserted just before the instruction."""
    cnt = 0
    for f in nc.m.functions:
        for bb in f.blocks:
            insts = list(bb.instructions)
            out = []
            changed = False
            for inst in insts:
                si = getattr(inst, "sync_info", None)
                ws = list(si.on_wait) if (si is not None and si.on_wait) else []
                if len(ws) > 1:
                    changed = True
                    for w in ws[:-1]:
                        cnt += 1
                        out.append(mybir.InstNoOp(
                            name=f"WSPLIT-{cnt}",
                            engine=inst.engine,
                            ins=[], outs=[],
                            sync_info=mybir.SyncInfo(on_wait=[w], on_update=[]),
                        ))
                    si.on_wait = [ws[-1]]
                    inst.sync_info = si
                out.append(inst)
            if changed:
                bb.instructions = out
    return nc


def _get_program(Asz, Bsz):
    key = (Asz, Bsz)
    if key not in _PROGRAM_CACHE:
        _PROGRAM_CACHE[key] = _split_multi_waits(_build(Asz, Bsz))
    return _PROGRAM_CACHE[key]


def _prep_inputs(features_a, features_b, W_qkv, Asz, Bsz, ncores):
    """Host-side: cast to bf16, fold the 1/sqrt(D) scale into Wq, reshape."""
    fa = features_a.reshape(Asz, HID, N).astype(ml_dtypes.bfloat16)
    fa = fa.reshape(Asz, KC, 128, N)
    wt = W_qkv.T.copy().astype(np.float32)   # [640, 240]
    wt[:, 0:D] *= SCALE
    wt = wt.astype(ml_dtypes.bfloat16).reshape(KC, 128, 240)
    eye = np.eye(128, dtype=ml_dtypes.bfloat16)
    fbs = []
    for c in range(ncores):
        fb = features_b[c * Bsz:(c + 1) * Bsz].reshape(Bsz, HID, N)
        fb = fb.astype(ml_dtypes.bfloat16).reshape(Bsz, KC, 128, N)
        fbs.append(fb)
    return fa, fbs, wt, eye


def kernel(features_a, features_b, W_qkv):
    Asz = features_a.shape[0]
    Bfull = features_b.shape[0]
    ncores = NCORES
    Bsz = Bfull // ncores
    fa, fbs, wt, eye = _prep_inputs(
        np.asarray(features_a), np.asarray(features_b), np.asarray(W_qkv),
        Asz, Bsz, ncores,
    )
    nc = _get_program(Asz, Bsz)
    in_maps = [{"fa": fa, "fb": fbs[c], "wt": wt, "eye": eye}
               for c in range(ncores)]
    res = run_bass_kernel_spmd(nc, in_maps, core_ids=list(range(ncores)))
    out = np.concatenate([res.results[c]["sim"].T for c in range(ncores)], axis=0)
    return out.astype(np.float32)


# revision 45
# speedup vs baseline: 2.3993x; 1.0034x over previous
"""Trainium2 Bass kernel for nn_CLARM_56693568307877.

Computes, for feature sets A [64,640,14,14] and B [128,640,14,14] and a QKV
projection W [240,640]:
    q,k,v = split(x^T W^T); S = q_b k_a^T / sqrt(80); P = softmax(S)
    rec = P v_a;  sim[b,a] = -||v_b - rec||^2_F
Output [128, 64] fp32.

Sharding: data-parallel over the b batch (16 per core x 8 cores);
features_a / W replicated. Device math in bf16 with fp32 accumulation
(max rel err ~3.4e-4 vs the fp32 reference).

Design (per-core B=16, A=64, N=196 tokens, D=80), n-major attention tail:
  phase 1 (one psum slot per batch, outside the subgroup windows):
    QKV on PE (W^T stationary, q|k and v packed into one bank), one ACT
    copy out d-major; v is PE-transposed (identity matmul, bf16 psum
    views packed in the same bank) into n-major chunk tiles
    va_cat/vb_cat [128/68 rows, chunk, batch, 81] with ones at col 80.
  phase 2, per (a, 4-b subgroup) on two alternating 4-slot psum windows:
    mm1  S^T chunks [mchunk, 392] for 4 pairs -> the window's 4 slots
    exp  one ACT instruction per m-chunk -> es bf16 SBUF
    mm2  U'[n, 4x81] = es^T @ [v_a|1] per n-chunk (den lands in col 80)
    tail w = 1/den (DVE, free-4); rec = U'*w via a stride-0 broadcast
         multiply (doubles as the psum egress, DVE); D = rec - vbn
         (DVE, 2x); D^2 on Pool; -sum over n via a (-1)-stationary
         matmul into a spare slot (row 0) of the own window; sum over d
         with one partition-1 reduce straight into the output row.
  The subgroup stages run as a 5-deep software pipeline (mm1+exp | mm2 +
  reciprocal/normalize, per n-chunk, sandwiched between the mm1 halves |
  subtract+square | n-reduce+d-reduce), so every engine's in-order queue
  only ever holds work whose inputs are already available, and the psum
  window of subgroup g is recycled by mm1(g+2) right after normalize(g).

Note: this walrus build accepts at most one semaphore wait per
instruction; _split_multi_waits hoists extras onto NoOps.
"""

import numpy as np
import ml_dtypes

import concourse.bass as bass
import concourse.tile as tile
from concourse import mybir
from concourse.bass_utils import run_bass_kernel_spmd

BF16 = mybir.dt.bfloat16
F32 = mybir.dt.float32

NCORES = 8
A_FULL = 64
B_FULL = 128
HID = 640
KC = HID // 128  # 5
N = 196          # tokens (14*14)
D = 80           # inner dim
MPAD = 256       # m padded to 2*128 so mm1 writes full psum rows
SCALE = 1.0 / np.sqrt(D)

_PROGRAM_CACHE = {}


def _build(Asz, Bsz):
    """Emit the Bass program for one core handling Bsz b's x Asz a's."""
    assert Bsz % 4 == 0 and Asz % 4 == 0
    NSG = Bsz // 4            # 4-b subgroups per a

    nc = bass.Bass("TRN2", debug=False)
    fa = nc.dram_tensor("fa", [Asz, KC, 128, N], BF16, kind="ExternalInput")
    fb = nc.dram_tensor("fb", [Bsz, KC, 128, N], BF16, kind="ExternalInput")
    wt = nc.dram_tensor("wt", [KC, 128, 240], BF16, kind="ExternalInput")
    eye = nc.dram_tensor("eye", [128, 128], BF16, kind="ExternalInput")
    simo = nc.dram_tensor("sim", [Asz, Bsz], F32, kind="ExternalOutput")

    Exp = mybir.ActivationFunctionType.Exp
    mult = mybir.AluOpType.mult
    sub = mybir.AluOpType.subtract

    with tile.TileContext(nc) as tc:
        with (
            tc.tile_pool(name="const", bufs=1) as cpool,
            tc.tile_pool(name="ring", bufs=1, space="PSUM") as rpool,
            tc.tile_pool(name="x", bufs=6) as x_pool,
            tc.tile_pool(name="e", bufs=5) as e_pool,
            tc.tile_pool(name="w", bufs=6) as w_pool,
            tc.tile_pool(name="rc", bufs=5) as rc_pool,
            tc.tile_pool(name="d", bufs=4) as d_pool,
            tc.tile_pool(name="sq", bufs=4) as sq_pool,
            tc.tile_pool(name="sr", bufs=3) as sr_pool,
        ):
            wt_sb = cpool.tile([128, KC, 240], BF16, tag="wt")
            eye_sb = cpool.tile([128, 128], BF16, tag="eye")
            # kv/qv: [d(128 zero-padded), batch, {q|k, v}, n] d-major
            kvT = cpool.tile([128, Asz, 2, N], BF16, tag="kvT")
            qvT = cpool.tile([128, Bsz, 2, N], BF16, tag="qvT")
            # n-major v with ones col: [n-chunk rows, chunk, batch, 81]
            va_cat = cpool.tile([128, 2, Asz, 81], BF16, tag="va")
            vb_cat = cpool.tile([128, 2, Bsz, 81], BF16, tag="vb")
            negones = cpool.tile([128, 1], BF16, tag="negones")
            ring = rpool.tile([128, 8, 512], F32, tag="ring")

            # one-time init
            nc.sync.dma_start(wt_sb, wt.ap().rearrange("k p c -> p k c"))
            nc.sync.dma_start(eye_sb, eye.ap())
            nc.gpsimd.memset(kvT[:], 0.0)
            nc.gpsimd.memset(qvT[:], 0.0)
            nc.gpsimd.memset(negones[:], -1.0)
            nc.gpsimd.memset(va_cat[:, :, :, 80:81], 1.0)
            nc.gpsimd.memset(vb_cat[:, :, :, 80:81], 1.0)
            nc.vector.memset(ring[:], 0.0)

            def qkv_mm(src, idx, want, wq):
                """Project one batch: q|k and v into one psum bank."""
                xt = x_pool.tile([128, KC, N], BF16, tag="x")
                nc.sync.dma_start(xt, src[idx].rearrange("k p n -> p k n"))
                c0 = 0 if want == "b" else 80   # q for b, k for a
                for kc in range(KC):
                    nc.tensor.matmul(ring[0:80, wq, 0:N],
                                     wt_sb[:, kc, c0:c0 + 80],
                                     xt[:, kc, :], start=(kc == 0),
                                     stop=(kc == KC - 1))
                for kc in range(KC):
                    nc.tensor.matmul(ring[0:80, wq, N:2 * N],
                                     wt_sb[:, kc, 160:240],
                                     xt[:, kc, :], start=(kc == 0),
                                     stop=(kc == KC - 1))

            def qkv_batch(src, idx, want, qkv_dst, v_cat, wq):
                """Copy the projections out (d-major), then PE-transpose the
                v half into n-major chunks and copy into v_cat."""
                nc.scalar.copy(qkv_dst, ring[0:80, wq, 0:2 * N])
                # transpose the v half into [n, d] chunks (bf16 psum views
                # packed in the SAME bank after the q|v block), then one
                # copy into v_cat — the whole batch occupies one psum slot
                # outside the subgroup windows' reuse path
                t0 = ring[0:128, wq, 392:432].bitcast(BF16)
                nc.tensor.matmul(t0, qvT[0:80, idx, 1, 0:128] if want == "b"
                                 else kvT[0:80, idx, 1, 0:128],
                                 eye_sb[0:80, 0:80],
                                 start=True, stop=True, is_transpose=True)
                t1 = ring[0:68, wq, 432:472].bitcast(BF16)
                nc.tensor.matmul(t1, qvT[0:80, idx, 1, 128:N] if want == "b"
                                 else kvT[0:80, idx, 1, 128:N],
                                 eye_sb[0:80, 0:80],
                                 start=True, stop=True, is_transpose=True)
                tcat = ring[0:128, wq, 392:472].bitcast(BF16)
                nc.scalar.copy(
                    v_cat[0:128, :, idx, 0:80],
                    tcat.rearrange("p (c d) -> p c d", c=2))

            def mm2_sg(p1, ncx):
                """Deferred mm2 for one n-chunk: U'[n, 4x81] accumulated
                over m-chunks; reuses slot base+ncx after exp read it."""
                (base, a, b0, simrow, dma_rows), es = p1
                ncols = 128 if ncx == 0 else 68
                for p in range(4):
                    for mc in range(2):
                        mr = 128 if mc == 0 else 68
                        nc.tensor.matmul(
                            ring[0:ncols, base + ncx, 81 * p:81 * p + 81],
                            es[0:mr, 2 * mc + p // 2,
                               (p % 2) * 196 + 128 * ncx:
                               (p % 2) * 196 + 128 * ncx + ncols],
                            va_cat[0:mr, mc, a, 0:81],
                            start=(mc == 0), stop=(mc == 1),
                        )

            def tail_recip(p, ncx):
                """reciprocal of one n-chunk's denominators (psum reader)."""
                ub, a, b0, simrow, dma_rows = p
                ncols = 128 if ncx == 0 else 68
                u3 = ring[0:ncols, ub + ncx, 0:324].rearrange(
                    "p (k d) -> p k d", k=4)
                den = u3[:, :, 80:81].rearrange("p k d -> p (k d)")
                w_t = w_pool.tile([128, 4], F32, tag="w")
                nc.vector.reciprocal(w_t[0:ncols, :], den)
                return w_t

            def tail_mult(p, ncx, w_t, rec):
                """fused normalize of one n-chunk (doubles as psum egress);
                emitted a few DVE ops after its reciprocal so the access
                ack of the reciprocal is hidden by independent work."""
                ub, a, b0, simrow, dma_rows = p
                ncols = 128 if ncx == 0 else 68
                u3 = ring[0:ncols, ub + ncx, 0:324].rearrange(
                    "p (k d) -> p k d", k=4)
                wbc = bass.AP(w_t.tensor, w_t.offset,
                              [[4, ncols], [1, 4], [0, 81]])
                nc.vector.tensor_tensor(rec[0:ncols, ncx, :, :], u3,
                                        wbc, op=mult)

            def mid_sub(p, rec):
                """SBUF-only subtract (DVE)."""
                ub, a, b0, simrow, dma_rows = p
                d_t = d_pool.tile([128, 2, 4, 81], BF16, tag="d")
                nc.vector.tensor_tensor(
                    d_t, rec, vb_cat[:, :, b0:b0 + 4, :], op=sub)
                return d_t

            def mid_sq(d_t):
                """SBUF-only square (Pool)."""
                sq_t = sq_pool.tile([128, 2, 4, 81], BF16, tag="sq")
                nc.gpsimd.tensor_tensor(sq_t, d_t, d_t, op=mult)
                return sq_t

            def fin_tail(p, sq_t):
                """-sum over n (matmul into a spare slot of the other
                window), then sum over d; output DMA after the last
                subgroup of a 4-a group."""
                ub, a, b0, simrow, dma_rows = p
                # slot 3 row 0 for every subgroup's n-reduce: its S^T
                # tenant's exp always precedes the nred in program order,
                # and slot 7 stays exclusively qkv's
                sim_ps = ring[0:1, 3, 0:324]
                nc.tensor.matmul(
                    sim_ps, negones[:, 0:1],
                    sq_t[0:128, 0, :, :].rearrange("p k d -> p (k d)"),
                    start=True, stop=False)
                nc.tensor.matmul(
                    sim_ps, negones[0:68, 0:1],
                    sq_t[0:68, 1, :, :].rearrange("p k d -> p (k d)"),
                    start=False, stop=True)
                nc.vector.reduce_sum(
                    out=simrow[0:1, a % 4, b0:b0 + 4],
                    in_=sim_ps.rearrange("p (k d) -> p k d", k=4),
                    axis=mybir.AxisListType.X,
                )
                if dma_rows is not None:
                    nc.sync.dma_start(simo[dma_rows[0]:dma_rows[1], :], simrow)

            # phase 1: the first 4 b batches up front; the rest are
            # interleaved into the early subgroup pipeline (subgroup sgb of
            # any a only needs b batches 4*sgb..4*sgb+3)
            # prologue: no subgroups exist yet, so fan the first four b
            # batches across four distinct psum slots
            for b in range(4):
                qkv_mm(fb, b, "b", 2 * b + 1)
            for b in range(4):
                qkv_batch(fb, b, "b", qvT[0:80, b, :, :], vb_cat, 2 * b + 1)

            pend1 = None  # awaiting early tail (deferred 1 subgroup)
            pend2 = None  # awaiting mid tail (deferred 2 subgroups)
            pend3 = None  # awaiting fin tail (deferred 3 subgroups)
            # first a's projections up front; later ones are prefetched
            # two subgroups before their a-boundary (kv-copy must precede
            # mm1 of the next a's first subgroup)
            qkv_mm(fa, 0, "a", 7)
            qkv_batch(fa, 0, "a", kvT[0:80, 0, :, :], va_cat, 7)
            for a in range(Asz):
                if a % 4 == 0:
                    simrow = sr_pool.tile([1, 4, Bsz], F32, tag="sr")

                for sgb in range(NSG):
                    b0 = 4 * sgb
                    # feed the remaining b batches during a=0: subgroup sgb
                    # needs b0..b0+3 of the NEXT subgroup ready one step
                    # ahead, so emit 4 per subgroup slot-alternating
                    if a == 0 and sgb < 3:
                        for bb in range(4 * sgb + 4, 4 * sgb + 8):
                            qkv_mm(fb, bb, "b", 3 + 4 * (bb % 2))
                            qkv_batch(fb, bb, "b", qvT[0:80, bb, :, :],
                                      vb_cat, 3 + 4 * (bb % 2))
                    if sgb == 2 and a + 1 < Asz:
                        qkv_mm(fa, a + 1, "a", 7)
                    if sgb == 3 and a + 1 < Asz:
                        qkv_batch(fa, a + 1, "a", kvT[0:80, a + 1, :, :],
                                  va_cat, 7)
                    base = 4 * ((a * NSG + sgb) % 2)
                    es = e_pool.tile([128, 4, 392], BF16, tag="e")
                    # 5-stage software pipeline over subgroups: mm2 + early
                    # tail at deferral 1, mid (sub+square) at 2, fin
                    # (n-reduce + d-reduce) at 3. mm1/exp are split per
                    # m-chunk and mm2(g-1) is emitted between the halves so
                    # the U'(g-1) -> mult(g-1) chain (which releases the
                    # next window) starts as early as possible while exp(g)
                    # still overlaps mm1(g).
                    for pp in range(2):
                        nc.tensor.matmul(
                            ring[0:128, base + pp, 0:392],
                            kvT[:, a, 0, 0:128],
                            qvT[:, b0 + 2 * pp:b0 + 2 * pp + 2, 0, :],
                            start=True, stop=True,
                        )
                    nc.scalar.activation(es[:, 0:2, :],
                                         ring[:, base:base + 2, 0:392], Exp)
                    nxt1 = None
                    if pend1 is not None:
                        rec = rc_pool.tile([128, 2, 4, 81], BF16, tag="rec")
                        mm2_sg(pend1, 0)
                        w0 = tail_recip(pend1[0], 0)
                        nxt1 = (pend1[0], rec)
                    d_t = mid_sub(*pend2) if pend2 is not None else None
                    if pend1 is not None:
                        tail_mult(pend1[0], 0, w0, nxt1[1])
                    nxt2 = (pend2[0], mid_sq(d_t)) if pend2 is not None \
                        else None
                    for pp in range(2):
                        nc.tensor.matmul(
                            ring[0:68, base + 2 + pp, 0:392],
                            kvT[:, a, 0, 128:196],
                            qvT[:, b0 + 2 * pp:b0 + 2 * pp + 2, 0, :],
                            start=True, stop=True,
                        )
                    nc.scalar.activation(es[:, 2:4, :],
                                         ring[:, base + 2:base + 4, 0:392], Exp)
                    if pend1 is not None:
                        mm2_sg(pend1, 1)
                        w1 = tail_recip(pend1[0], 1)
                    if pend3 is not None:
                        fin_tail(*pend3)
                    if pend1 is not None:
                        tail_mult(pend1[0], 1, w1, nxt1[1])
                    pend3, pend2 = nxt2, nxt1
                    dma_rows = (a - 3, a + 1) if (a % 4 == 3 and
                                                  sgb == NSG - 1) else None
                    pend1 = ((base, a, b0, simrow, dma_rows), es)
            if pend1 is not None:
                rec = rc_pool.tile([128, 2, 4, 81], BF16, tag="rec")
                mm2_sg(pend1, 0)
                tail_mult(pend1[0], 0, tail_recip(pend1[0], 0), rec)
                mm2_sg(pend1, 1)
                tail_mult(pend1[0], 1, tail_recip(pend1[0], 1), rec)
                pend0 = (pend1[0], rec)
            else:
                pend0 = None
            if pend3 is not None:
                fin_tail(*pend3)
            if pend2 is not None:
                fin_tail(pend2[0], mid_sq(mid_sub(*pend2)))
            if pend0 is not None:
                fin_tail(pend0[0], mid_sq(mid_sub(*pend0)))

    return nc


def _split_multi_waits(nc):
    """This walrus build accepts at most one semaphore wait per instruction;
    Tile emits several (incl. its tail drain). Hoist extra waits onto
    single-wait engine NoOps inserted just before the instruction."""
    cnt = 0
    for f in nc.m.functions:
        for bb in f.blocks:
            insts = list(bb.instructions)
            out = []
            changed = False
            for inst in insts:
                si = getattr(inst, "sync_info", None)
                ws = list(si.on_wait) if (si is not None and si.on_wait) else []
                if len(ws) > 1:
                    changed = True
                    for w in ws[:-1]:
                        cnt += 1
                        out.append(mybir.InstNoOp(
                            name=f"WSPLIT-{cnt}",
                            engine=inst.engine,
                            ins=[], outs=[],
                            sync_info=mybir.SyncInfo(on_wait=[w], on_update=[]),
                        ))
                    si.on_wait = [ws[-1]]
                    inst.sync_info = si
                out.append(inst)
            if changed:
                bb.instructions = out
    return nc


def _get_program(Asz, Bsz):
    key = (Asz, Bsz)
    if key not in _PROGRAM_CACHE:
        _PROGRAM_CACHE[key] = _split_multi_waits(_build(Asz, Bsz))
    return _PROGRAM_CACHE[key]


def _prep_inputs(features_a, features_b, W_qkv, Asz, Bsz, ncores):
    """Host-side: cast to bf16, fold the 1/sqrt(D) scale into Wq, reshape."""
    fa = features_a.reshape(Asz, HID, N).astype(ml_dtypes.bfloat16)
    fa = fa.reshape(Asz, KC, 128, N)
    wt = W_qkv.T.copy().astype(np.float32)   # [640, 240]
    wt[:, 0:D] *= SCALE
    wt = wt.astype(ml_dtypes.bfloat16).reshape(KC, 128, 240)
    eye = np.eye(128, dtype=ml_dtypes.bfloat16)
    fbs = []
    for c in range(ncores):
        fb = features_b[c * Bsz:(c + 1) * Bsz].reshape(Bsz, HID, N)
        fb = fb.astype(ml_dtypes.bfloat16).reshape(Bsz, KC, 128, N)
        fbs.append(fb)
    return fa, fbs, wt, eye


def kernel(features_a, features_b, W_qkv):
    Asz = features_a.shape[0]
    Bfull = features_b.shape[0]
    ncores = NCORES
    Bsz = Bfull // ncores
    fa, fbs, wt, eye = _prep_inputs(
        np.asarray(features_a), np.asarray(features_b), np.asarray(W_qkv),
        Asz, Bsz, ncores,
    )
    nc = _get_program(Asz, Bsz)
    in_maps = [{"fa": fa, "fb": fbs[c], "wt": wt, "eye": eye}
               for c in range(ncores)]
    res = run_bass_kernel_spmd(nc, in_maps, core_ids=list(range(ncores)))
    out = np.concatenate([res.results[c]["sim"].T for c in range(ncores)], axis=0)
    return out.astype(np.float32)
